# revision 10
# baseline (speedup 1.0000x reference)
"""Trainium2 Bass kernel for nn_CausalAdapter.

Data-parallel over B across 8 NeuronCores (8 batch rows per core), all
weights replicated. The big v = tok_cause @ v_w matmul is algebraically
eliminated: bias[b,h,k] = sum_D tok_cause[b,k,D] * wtil[b,h,D] where
wtil[b,h,D] = sum_d u[b,h,d] * v_w[D, h*128+d]. logit_bias is a broadcast
of bias over NQ, written with a stride-0 replicating DMA.
"""

import numpy as np

B, D, H, DH, NQ, NK = 64, 768, 8, 128, 256, 256
NCORES = 8
BS = B // NCORES  # 8 rows per core
P = 128
KC2D = (2 * D) // P  # 12 contraction chunks for 2D=1536
KCD = D // P         # 6 chunks for D=768
RSQRT_DH = 1.0 / np.sqrt(np.float32(DH))

_CACHE = {}


def _build():
    from contextlib import ExitStack

    import concourse.bass as bass
    import concourse.mybir as mybir
    import concourse.tile as tile
    from concourse.masks import make_identity

    fp32 = mybir.dt.float32
    Alu = mybir.AluOpType
    Act = mybir.ActivationFunctionType

    nc = bass.Bass("TRN2", target_bir_lowering=False, debug=False)

    def din(name, shape):
        return nc.dram_tensor(name, shape, fp32, kind="ExternalInput").ap()

    def dout(name, shape):
        return nc.dram_tensor(name, shape, fp32, kind="ExternalOutput").ap()

    emb_c = din("emb_cause", [BS, D])
    emb_a = din("emb_action", [BS, D])
    tok_c = din("tok_cause", [BS, NK, D])
    tok_e = din("tok_effect", [BS, NQ, D])
    fuse_ln_g = din("fuse_ln_g", [2 * D])
    fuse_ln_b = din("fuse_ln_b", [2 * D])
    fuse_w1 = din("fuse_w1", [2 * D, D])
    fuse_b1 = din("fuse_b1", [D])
    fuse_w2 = din("fuse_w2", [D, D])
    fuse_b2 = din("fuse_b2", [D])
    delta_ln_g = din("delta_ln_g", [D])
    delta_ln_b = din("delta_ln_b", [D])
    delta_w = din("delta_w", [D, D])
    delta_b = din("delta_b", [D])
    qg_w = din("qg_w", [D, H * DH])
    qg_b = din("qg_b", [H * DH])
    kg_w = din("kg_w", [D, H * DH])
    kg_b = din("kg_b", [H * DH])
    eff_ln_g = din("eff_ln_g", [D])
    eff_ln_b = din("eff_ln_b", [D])
    eff_w1 = din("eff_w1", [D, D])
    eff_b1 = din("eff_b1", [D])
    eff_w2 = din("eff_w2", [D, D])
    eff_b2 = din("eff_b2", [D])
    u_w = din("u_w", [D, H * DH])
    v_w = din("v_w", [D, H * DH])
    logit_scale = din("logit_scale", [1])
    pooled_mask = din("pooled_mask", [P, BS, BS])  # [p, b, m] = (m==b)/NQ

    q_gate_o = dout("q_gate", [BS, H * DH])
    k_gate_o = dout("k_gate", [BS, H * DH])
    e_shifted_o = dout("e_shifted", [BS, D])
    delta_e_o = dout("delta_e", [BS, D])
    effect_pred_o = dout("effect_pred", [BS, D])
    logit_bias_o = dout("logit_bias", [BS, H, NQ, NK])

    def bcast_rows(ap1d, rows):
        # replicate a 1-D DRAM vector across `rows` partitions (step-0 AP)
        return bass.AP(tensor=ap1d.tensor, offset=ap1d.offset,
                       ap=[[0, rows], *ap1d.ap])

    with ExitStack() as ctx:
        tc = ctx.enter_context(tile.TileContext(nc))
        consts = ctx.enter_context(tc.tile_pool(name="consts", bufs=1))
        reps = ctx.enter_context(tc.tile_pool(name="reps", bufs=1))
        sm = ctx.enter_context(tc.tile_pool(name="sm", bufs=1))
        wpool = ctx.enter_context(tc.tile_pool(name="wpool", bufs=3))
        vwtp = ctx.enter_context(tc.tile_pool(name="vwtp", bufs=1))
        tep = ctx.enter_context(tc.tile_pool(name="tep", bufs=3))
        tcp = ctx.enter_context(tc.tile_pool(name="tcp", bufs=3))
        toktp = ctx.enter_context(tc.tile_pool(name="toktp", bufs=2))
        biasp = ctx.enter_context(tc.tile_pool(name="biasp", bufs=2))
        ps_tr = ctx.enter_context(tc.tile_pool(name="ps_tr", bufs=2, space="PSUM"))
        ps_mm = ctx.enter_context(tc.tile_pool(name="ps_mm", bufs=2, space="PSUM"))
        ps_pool = ctx.enter_context(tc.tile_pool(name="ps_pool", bufs=1, space="PSUM"))

        ident = consts.tile([P, P], fp32)
        make_identity(nc, ident)
        eps8 = consts.tile([BS, 1], fp32)
        nc.vector.memset(eps8, 1e-5)
        ones8 = consts.tile([1, BS], fp32)
        nc.vector.memset(ones8, RSQRT_DH)
        mask_sb = consts.tile([P, BS, BS], fp32)
        nc.sync.dma_start(out=mask_sb, in_=pooled_mask)

        # replicated per-column vectors (step-0 broadcast loads)
        def rep_tile(src, width):
            t = reps.tile([BS, width], fp32)
            nc.gpsimd.dma_start(out=t, in_=bcast_rows(src, BS))
            return t

        g1r = rep_tile(fuse_ln_g, 2 * D)
        b1r = rep_tile(fuse_ln_b, 2 * D)
        fb1r = rep_tile(fuse_b1, D)
        fb2r = rep_tile(fuse_b2, D)
        dgr = rep_tile(delta_ln_g, D)
        dbr = rep_tile(delta_ln_b, D)
        d_br = rep_tile(delta_b, D)
        egr = rep_tile(eff_ln_g, D)
        ebr = rep_tile(eff_ln_b, D)
        eb1r = rep_tile(eff_b1, D)
        eb2r = rep_tile(eff_b2, D)
        qgbr = rep_tile(qg_b, H * DH)
        kgbr = rep_tile(kg_b, H * DH)

        # ---- logit scale: s = exp(logit_scale)/sqrt(DH), replicated to 8 rows
        ls_sb = sm.tile([1, 1], fp32, tag="ls")
        nc.sync.dma_start(out=ls_sb, in_=bass.AP(
            tensor=logit_scale.tensor, offset=logit_scale.offset,
            ap=[[0, 1], [1, 1]]))
        exp_sb = sm.tile([1, 1], fp32, tag="exp")
        nc.scalar.activation(exp_sb, ls_sb, Act.Exp)
        s_ps = ps_mm.tile([BS, 1], fp32, tag="mm")
        nc.tensor.matmul(s_ps, lhsT=ones8, rhs=exp_sb, start=True, stop=True)
        s_rep = sm.tile([BS, 1], fp32, tag="srep")
        nc.any.tensor_copy(out=s_rep, in_=s_ps)

        # ---- v_w load + transpose -> vwT[p=d, h, D] = v_w[D, h*128+d]
        vwT = vwtp.tile([P, H, D], fp32)
        for dc in range(KCD):
            vwn = wpool.tile([P, H * DH], fp32, tag="w")
            nc.sync.dma_start(
                out=vwn, in_=v_w.rearrange("(c p) n -> p c n", p=P)[:, dc, :])
            for h in range(H):
                pst = ps_tr.tile([P, P], fp32, tag="tr")
                nc.tensor.transpose(pst, vwn[:, h * DH:(h + 1) * DH], ident)
                nc.any.tensor_copy(out=vwT[:, h, dc * P:(dc + 1) * P], in_=pst)

        # helper: transpose natural [BS, width] -> dst [P, width//P, BS]
        def transpose_rows(src_sb, width, dst):
            for c in range(width // P):
                pst = ps_tr.tile([P, P], fp32, tag="tr")
                nc.tensor.transpose(pst[:, :BS], src_sb[:, c * P:(c + 1) * P],
                                    ident[:BS, :BS])
                nc.any.tensor_copy(out=dst[:, c, :], in_=pst[:, :BS])

        # helper: layernorm core -> writes normalized (x-m)*rstd into dst
        def ln_core(x_sb, width, dst, tag):
            sub = 512 if width % 512 == 0 else 256
            nsub = width // sub
            stats = sm.tile([BS, nsub, 6], fp32, tag=f"st_{tag}")
            for i in range(nsub):
                nc.vector.bn_stats(out=stats[:, i, :],
                                   in_=x_sb[:, i * sub:(i + 1) * sub])
            mv = sm.tile([BS, 2], fp32, tag=f"mv_{tag}")
            nc.vector.bn_aggr(out=mv, in_=stats)
            std = sm.tile([BS, 1], fp32, tag=f"sd_{tag}")
            nc.scalar.activation(std, mv[:, 1:2], Act.Sqrt, bias=eps8)
            rstd = sm.tile([BS, 1], fp32, tag=f"rs_{tag}")
            nc.vector.reciprocal(rstd, std)
            nc.vector.tensor_scalar(out=dst, in0=x_sb, scalar1=mv[:, 0:1],
                                    scalar2=rstd, op0=Alu.subtract, op1=Alu.mult)

        # ---- fuse path
        cin = sm.tile([BS, 2 * D], fp32, tag="cin")
        nc.sync.dma_start(out=cin[:, :D], in_=emb_c)
        nc.sync.dma_start(out=cin[:, D:], in_=emb_a)
        ln1 = sm.tile([BS, 2 * D], fp32, tag="ln1")
        ln_core(cin, 2 * D, ln1, "ln1")
        nc.vector.tensor_mul(ln1, ln1, g1r)
        nc.vector.tensor_add(ln1, ln1, b1r)
        ln1T = sm.tile([P, KC2D, BS], fp32, tag="ln1T")
        transpose_rows(ln1, 2 * D, ln1T)

        # matmul natural: out[BS, width] = aT.T @ w + bias, w streamed by k-chunk
        def mm_natural(aT, nk, w_dram, width, add_bias, act, dst, tag):
            w_r = w_dram.rearrange("(c p) n -> p c n", p=P)
            nchunks = [(n0, min(512, width - n0)) for n0 in range(0, width, 512)]
            pss = []
            for i, (n0, nw) in enumerate(nchunks):
                ps_full = ps_mm.tile([BS, 512], fp32, tag="mm", name=f"mm_{tag}_{i}")
                pss.append(ps_full[:, :nw])
            for kc in range(nk):
                wt_full = wpool.tile([P, 1024], fp32, tag="w", name=f"w_{tag}_{kc}")
                wt = wt_full[:, :width]
                nc.sync.dma_start(out=wt, in_=w_r[:, kc, :])
                for i, (n0, nw) in enumerate(nchunks):
                    nc.tensor.matmul(pss[i], lhsT=aT[:, kc, :],
                                     rhs=wt[:, n0:n0 + nw],
                                     start=(kc == 0), stop=(kc == nk - 1))
            for i, (n0, nw) in enumerate(nchunks):
                ps = pss[i]
                if add_bias is not None:
                    nc.vector.tensor_add(dst[:, n0:n0 + nw], ps,
                                         add_bias[:, n0:n0 + nw])
                    if act is not None:
                        nc.scalar.activation(dst[:, n0:n0 + nw],
                                             dst[:, n0:n0 + nw], act)
                elif act is not None:
                    nc.scalar.activation(dst[:, n0:n0 + nw], ps, act)
                else:
                    nc.any.tensor_copy(out=dst[:, n0:n0 + nw], in_=ps)

        h_sb = sm.tile([BS, D], fp32, tag="h")
        mm_natural(ln1T, KC2D, fuse_w1, D, fb1r, Act.Gelu, h_sb, "h")
        hT = sm.tile([P, KCD, BS], fp32, tag="hT")
        transpose_rows(h_sb, D, hT)

        ctx_sb = sm.tile([BS, D], fp32, tag="ctx")
        mm_natural(hT, KCD, fuse_w2, D, fb2r, None, ctx_sb, "ctx")

        # ---- shared LN(ctx), delta/eff branches
        nrm = sm.tile([BS, D], fp32, tag="nrm")
        ln_core(ctx_sb, D, nrm, "ln2")
        dln = sm.tile([BS, D], fp32, tag="dln")
        nc.vector.tensor_mul(dln, nrm, dgr)
        nc.vector.tensor_add(dln, dln, dbr)
        eln = sm.tile([BS, D], fp32, tag="eln")
        nc.vector.tensor_mul(eln, nrm, egr)
        nc.vector.tensor_add(eln, eln, ebr)
        dlnT = sm.tile([P, KCD, BS], fp32, tag="dlnT")
        transpose_rows(dln, D, dlnT)
        elnT = sm.tile([P, KCD, BS], fp32, tag="elnT")
        transpose_rows(eln, D, elnT)
        ctxT = sm.tile([P, KCD, BS], fp32, tag="ctxT")
        transpose_rows(ctx_sb, D, ctxT)

        delta_e_sb = sm.tile([BS, D], fp32, tag="de")
        mm_natural(dlnT, KCD, delta_w, D, d_br, None, delta_e_sb, "de")
        nc.sync.dma_start(out=delta_e_o, in_=delta_e_sb)

        # ---- pooled_effect accumulated in PSUM via one-hot masks
        pool_a = ps_pool.tile([BS, 512], fp32, tag="pa")
        pool_b = ps_pool.tile([BS, 256], fp32, tag="pb")
        for b in range(BS):
            te_b = tep.tile([P, 2, D], fp32, tag="te")
            nc.sync.dma_start(out=te_b,
                              in_=tok_e[b].rearrange("(kc p) d -> p kc d", p=P))
            for kc in range(2):
                st = (b == 0 and kc == 0)
                sp = (b == BS - 1 and kc == 1)
                nc.tensor.matmul(pool_a, lhsT=mask_sb[:, b, :],
                                 rhs=te_b[:, kc, 0:512], start=st, stop=sp)
                nc.tensor.matmul(pool_b, lhsT=mask_sb[:, b, :],
                                 rhs=te_b[:, kc, 512:768], start=st, stop=sp)

        esh = sm.tile([BS, D], fp32, tag="esh")
        nc.vector.tensor_add(esh[:, 0:512], delta_e_sb[:, 0:512], pool_a)
        nc.vector.tensor_add(esh[:, 512:768], delta_e_sb[:, 512:768], pool_b)
        nc.sync.dma_start(out=e_shifted_o, in_=esh)
        eshT = sm.tile([P, KCD, BS], fp32, tag="eshT")
        transpose_rows(esh, D, eshT)

        # ---- u = e_shifted @ u_w  (no bias), then uT
        u_sb = sm.tile([BS, H * DH], fp32, tag="u")
        mm_natural(eshT, KCD, u_w, H * DH, None, None, u_sb, "u")
        uT = sm.tile([P, H, BS], fp32, tag="uT")
        transpose_rows(u_sb, H * DH, uT)

        # ---- wtil[b,h,D]: wtT[p=Dp, dc, b, h]
        wtT = sm.tile([P, KCD, BS, H], fp32, tag="wtT")
        for h in range(H):
            for dc in range(KCD):
                wps = ps_mm.tile([P, BS], fp32, tag="mm")
                nc.tensor.matmul(wps, lhsT=vwT[:, h, dc * P:(dc + 1) * P],
                                 rhs=uT[:, h, :], start=True, stop=True)
                nc.any.tensor_copy(out=wtT[:, dc, :, h], in_=wps)

        # ---- effect_pred path (independent leaf)
        eh_sb = sm.tile([BS, D], fp32, tag="eh")
        mm_natural(elnT, KCD, eff_w1, D, eb1r, Act.Gelu, eh_sb, "eh")
        ehT = sm.tile([P, KCD, BS], fp32, tag="ehT")
        transpose_rows(eh_sb, D, ehT)
        ep_sb = sm.tile([BS, D], fp32, tag="ep")
        mm_natural(ehT, KCD, eff_w2, D, eb2r, None, ep_sb, "ep")
        nc.sync.dma_start(out=effect_pred_o, in_=ep_sb)

        # ---- gates
        def gate(wt_dram, b_rep, out_dram, tag):
            gl = sm.tile([BS, H * DH], fp32, tag=f"g_{tag}")
            mm_natural(ctxT, KCD, wt_dram, H * DH, b_rep, Act.Silu, gl, tag)
            nc.scalar.activation(gl, gl, Act.Tanh)
            nc.vector.tensor_scalar_add(gl, gl, 1.0)
            nc.sync.dma_start(out=out_dram, in_=gl)

        gate(qg_w, qgbr, q_gate_o, "qg")
        gate(kg_w, kgbr, k_gate_o, "kg")

        # ---- per-b: transpose tok_cause, bias matmul, broadcast write
        for b in range(BS):
            tc_b = tcp.tile([P, 2, D], fp32, tag="tc")
            nc.sync.dma_start(out=tc_b,
                              in_=tok_c[b].rearrange("(kc p) d -> p kc d", p=P))
            tokT_b = toktp.tile([P, KCD, NK], fp32, tag="tokT")
            for kc in range(2):
                for dc in range(KCD):
                    pst = ps_tr.tile([P, P], fp32, tag="tr")
                    nc.tensor.transpose(pst, tc_b[:, kc, dc * P:(dc + 1) * P],
                                        ident)
                    nc.any.tensor_copy(
                        out=tokT_b[:, dc, kc * P:(kc + 1) * P], in_=pst)
            bps = ps_mm.tile([BS, NK], fp32, tag="mm")
            for dc in range(KCD):
                nc.tensor.matmul(bps, lhsT=wtT[:, dc, b, :], rhs=tokT_b[:, dc, :],
                                 start=(dc == 0), stop=(dc == KCD - 1))
            bias_b = biasp.tile([BS, NK], fp32, tag="bias")
            nc.vector.tensor_scalar_mul(bias_b, bps, s_rep)
            # broadcast over NQ: stride-0 source AP, 2.1 MiB contiguous dest
            src = bass.AP(tensor=bias_b.tensor, offset=bias_b.offset,
                          ap=[bias_b.ap[0], [0, NQ], *bias_b.ap[1:]])
            nc.gpsimd.dma_start(out=logit_bias_o[b], in_=src)

    _legalize_waits(nc)
    return nc


def _legalize_waits(nc):
    """This walrus build allows at most ONE sync wait attached per
    instruction. Hoist extra waits onto standalone InstEventSemaphore ops
    on the same engine queue (they execute before the instruction)."""
    import copy

    import concourse.mybir as mybir

    m = nc.m
    new_module = copy.replace(m, functions=[])
    for function in m.functions:
        new_function = copy.replace(function, blocks=[])
        new_function.set_allocations_from_list(function.allocations)
        for block in function.blocks:
            out = []
            for inst in block.instructions:
                si = inst.sync_info
                waits = list(si.on_wait) if si is not None and si.on_wait else []
                if len(waits) > 1:
                    for w in waits[:-1]:
                        ev = mybir.InstEventSemaphore(
                            name=f"LW-{nc.next_id()}", ins=[], outs=[],
                            engine=inst.engine,
                            sync_info=mybir.SyncInfo(on_wait=[w], on_update=[]))
                        out.append(ev)
                    inst.sync_info = mybir.SyncInfo(
                        on_wait=[waits[-1]],
                        on_update=list(si.on_update) if si.on_update else [])
                out.append(inst)
            new_block = copy.replace(block, instructions=out)
            new_function.blocks.append(new_block)
        new_module.functions.append(new_function)
    nc.m = new_module


def _get_nc():
    if "nc" not in _CACHE:
        _CACHE["nc"] = _build()
    return _CACHE["nc"]


def kernel(**inputs):
    from concourse.bass_utils import run_bass_kernel_spmd

    nc = _get_nc()
    inp = {k: np.ascontiguousarray(np.asarray(v, dtype=np.float32))
           for k, v in inputs.items()}

    mask = np.zeros((P, BS, BS), dtype=np.float32)
    for b in range(BS):
        mask[:, b, b] = 1.0 / NQ

    in_maps = []
    for c in range(NCORES):
        sl = slice(c * BS, (c + 1) * BS)
        m = {
            "emb_cause": inp["emb_cause"][sl],
            "emb_action": inp["emb_action"][sl],
            "tok_cause": inp["tok_cause"][sl],
            "tok_effect": inp["tok_effect"][sl],
            "logit_scale": inp["logit_scale"].reshape(1),
            "pooled_mask": mask,
        }
        for k in ("fuse_ln_g", "fuse_ln_b", "fuse_w1", "fuse_b1", "fuse_w2",
                  "fuse_b2", "delta_ln_g", "delta_ln_b", "delta_w", "delta_b",
                  "qg_w", "qg_b", "kg_w", "kg_b", "eff_ln_g", "eff_ln_b",
                  "eff_w1", "eff_b1", "eff_w2", "eff_b2", "u_w", "v_w"):
            m[k] = inp[k]
        in_maps.append(m)

    r = run_bass_kernel_spmd(nc, in_maps, list(range(NCORES)),
                             trace=_CACHE.get("trace", False))
    _CACHE["last"] = r
    res = r.results

    def cat(name):
        return np.concatenate([res[c][name] for c in range(NCORES)], axis=0)

    q_gate = cat("q_gate").reshape(B, H, DH)
    k_gate = cat("k_gate").reshape(B, H, DH)
    e_shifted = cat("e_shifted")
    delta_e = cat("delta_e")
    logit_bias = cat("logit_bias")
    effect_pred = cat("effect_pred")
    return (q_gate, k_gate, e_shifted, delta_e, logit_bias, effect_pred)
